# revision 7
# baseline (speedup 1.0000x reference)
"""Trainium2 Bass kernel for nn_LoRAPool (MoE top-2 LoRA expert pool).

Math (reference):
    gates[t,e] = p_L[t,e] if e in top-2 of p_L[t,:] else 0
    hr[t,e,r]  = sum_d h[t,d] * A[e,r,d]
    out[t,d]   = sum_{e,r} hr[t,e,r] * 2.0 * gates[t,e] * B[e,d,r]

Folded into two dense matmuls over c = (e,r) in [0,128):
    A_cat[d,c] = 2.0 * A[e,r,d];  B_cat[c,d] = B[e,d,r]
    U^T[c,t]   = sum_d A_cat[d,c] hT[d,t]       (stage 1, PE, bf16)
    Us[c,t]    = U^T[c,t] * gates[t, c//16]     (gating, DVE, f32-exact gates)
    out[t,d]   = sum_c Us[c,t] B_cat[c,d]       (stage 2, PE, bf16)

Memory-bound: all large traffic (h in, out) is bf16 (tolerance 2e-2; bf16
end-to-end error is ~5e-3). h is pre-transposed AND pre-tiled on the host
([p, group, k, t] layout) so every device DMA has 8-16KB contiguous lines
and no on-device transposes are needed. Token groups of 512 pipeline:
group g's output store overlaps group g+1's input stream.

Sharding: tokens (4*4096 = 16384) split evenly across 8 cores; weights and
helper matrices replicated.
"""

import numpy as np

N_CORES = 8
B_SZ, S_SZ, D = 4, 4096, 2048
E, R, C = 8, 16, 128
T_FULL = B_SZ * S_SZ            # 16384 tokens
T_CORE = T_FULL // N_CORES      # 2048 tokens per core
GROUP = 512                     # token group (stage-1 PSUM bank width)
N_GROUPS = T_CORE // GROUP      # 4
N_SUBTOT = T_CORE // 128        # 16 sub-blocks of 128 tokens per core
SUB_PER_GROUP = GROUP // 128    # 4
KD = D // 128                   # 16 contraction chunks
KH = KD // 2                    # chunks per hT half-DMA
SCALING = 2.0

_CACHE = {}


def _build_nc(split_waits=True):
    import concourse.bass as bass
    import concourse.tile as tile
    import concourse.mybir as mybir
    from contextlib import ExitStack

    f32 = mybir.dt.float32
    f32r = mybir.dt.float32r
    bf16 = mybir.dt.bfloat16

    nc = bass.Bass()
    # hT[p, g*KD*GROUP + k*GROUP + t] = h[token g*GROUP+t, d = k*128+p]
    ht_d = nc.declare_dram_parameter(
        "hT", [128, N_GROUPS * KD * GROUP], bf16, isOutput=False
    )
    p_d = nc.declare_dram_parameter("p_perm", [T_CORE, E], f32, isOutput=False)
    a_d = nc.declare_dram_parameter("A_cat", [128, KD * C], bf16, isOutput=False)
    b_d = nc.declare_dram_parameter("B_cat", [C, D], bf16, isOutput=False)
    m_d = nc.declare_dram_parameter("Mexp", [E, C], f32, isOutput=False)
    i_d = nc.declare_dram_parameter("Ident", [128, 128], f32, isOutput=False)
    o_d = nc.declare_dram_parameter("out", [T_CORE, D], bf16, isOutput=True)

    AX = mybir.AxisListType
    OP = mybir.AluOpType

    with ExitStack() as ctx:
        tc = ctx.enter_context(tile.TileContext(nc))
        consts = ctx.enter_context(tc.tile_pool(name="consts", bufs=1))
        hpool = ctx.enter_context(tc.tile_pool(name="h", bufs=2 * N_GROUPS))
        gpool = ctx.enter_context(tc.tile_pool(name="gates", bufs=1))
        gtpool = ctx.enter_context(tc.tile_pool(name="gt", bufs=2))
        gsbpool = ctx.enter_context(tc.tile_pool(name="gsb", bufs=2))
        utspool = ctx.enter_context(tc.tile_pool(name="uts", bufs=2))
        opool = ctx.enter_context(tc.tile_pool(name="osb", bufs=3))
        ps_u = ctx.enter_context(tc.tile_pool(name="ps_u", bufs=2, space="PSUM"))
        ps_g = ctx.enter_context(tc.tile_pool(name="ps_g", bufs=2, space="PSUM"))
        ps_o = ctx.enter_context(tc.tile_pool(name="ps_o", bufs=2, space="PSUM"))

        # ---- constants + routing probs first (small, clears the queue) ----
        A_sb = consts.tile([128, KD * C], bf16)
        nc.sync.dma_start(out=A_sb, in_=a_d[:, :])
        p_sb = gpool.tile([128, N_SUBTOT, E], f32)
        nc.sync.dma_start(out=p_sb, in_=p_d.rearrange("(p n) e -> p n e", p=128))
        I_sb = consts.tile([128, 128], f32)
        nc.sync.dma_start(out=I_sb, in_=i_d[:, :])
        M_raw = consts.tile([E, C], f32)
        nc.sync.dma_start(out=M_raw, in_=m_d[:, :])
        M_sb = consts.tile([E, C], f32r)
        nc.vector.tensor_copy(out=M_sb, in_=M_raw)

        # ---- top-2 gates for the whole core: [128 tok, 16 sub, 8 exp] ----
        m1 = gpool.tile([128, N_SUBTOT, 1], f32)
        nc.vector.tensor_reduce(out=m1, in_=p_sb, axis=AX.X, op=OP.max)
        mlt = gpool.tile([128, N_SUBTOT, E], f32)
        nc.vector.tensor_tensor(
            out=mlt, in0=p_sb, in1=m1.broadcast_to([128, N_SUBTOT, E]), op=OP.is_lt
        )
        pm = gpool.tile([128, N_SUBTOT, E], f32)
        nc.vector.tensor_mul(pm, p_sb, mlt)
        m2 = gpool.tile([128, N_SUBTOT, 1], f32)
        nc.vector.tensor_reduce(out=m2, in_=pm, axis=AX.X, op=OP.max)
        ge2 = gpool.tile([128, N_SUBTOT, E], f32)
        nc.vector.tensor_tensor(
            out=ge2, in0=p_sb, in1=m2.broadcast_to([128, N_SUBTOT, E]), op=OP.is_ge
        )
        gts = gpool.tile([128, N_SUBTOT, E], f32)
        nc.vector.tensor_mul(gts, p_sb, ge2)

        B_issued = [False]

        def issue_ht(g):
            tiles = []
            for h2 in range(2):
                ht = hpool.tile([128, KH, GROUP], bf16, tag="h", name=f"ht{g}_{h2}")
                off = g * KD * GROUP + h2 * KH * GROUP
                nc.sync.dma_start(
                    out=ht,
                    in_=ht_d[:, off : off + KH * GROUP].rearrange(
                        "p (k t) -> p k t", k=KH
                    ),
                )
                tiles.append(ht)
            return tiles

        # issue ALL h loads before any output store enters the (in-order)
        # sync queue — otherwise stores head-of-line block later h streams
        ht_tiles = {0: issue_ht(0)}
        B_sb = consts.tile([C, D], bf16)
        nc.sync.dma_start(out=B_sb, in_=b_d[:, :])
        for g in range(1, N_GROUPS):
            ht_tiles[g] = issue_ht(g)

        copy_flip = 0
        for g in range(N_GROUPS):
            # dense gate matrix G[c, t] for this group
            gt_ps = ps_g.tile([128, GROUP], f32, tag="g", name=f"gt{g}")
            for s4 in range(SUB_PER_GROUP):
                s = g * SUB_PER_GROUP + s4
                nc.tensor.transpose(
                    out=gt_ps[:E, s4 * 128 : (s4 + 1) * 128],
                    in_=gts[:, s, :],
                    identity=I_sb,
                )
            gt_sb = gtpool.tile([E, GROUP], f32r, tag="gtsb")
            nc.vector.tensor_copy(out=gt_sb, in_=gt_ps[:E, :])
            G_ps = ps_g.tile([128, GROUP], f32, tag="g", name=f"G{g}")
            nc.tensor.matmul(G_ps, lhsT=M_sb, rhs=gt_sb, start=True, stop=True)
            G_sb = gsbpool.tile([128, GROUP], f32, tag="gsb")
            nc.scalar.copy(out=G_sb, in_=G_ps)

            # stage 1: U^T[c, t] accumulated over 16 d-chunks
            U_ps = ps_u.tile([128, GROUP], f32, tag="u", name=f"U{g}")
            for k in range(KD):
                nc.tensor.matmul(
                    U_ps,
                    lhsT=A_sb[:, k * C : (k + 1) * C],
                    rhs=ht_tiles[g][k // KH][:, k % KH, :],
                    start=(k == 0),
                    stop=(k == KD - 1),
                )

            # gating
            uts = utspool.tile([128, GROUP], bf16, tag="uts")
            nc.vector.tensor_tensor(out=uts, in0=U_ps, in1=G_sb, op=OP.mult)

            # stage 2 + store, per 128-token sub-tile
            for s4 in range(SUB_PER_GROUP):
                s = g * SUB_PER_GROUP + s4
                o_sb = opool.tile([128, D], bf16, tag="osb")
                for jh in range(2):
                    o_ps = ps_o.tile([128, 1024], f32, tag="o", name=f"o{s}_{jh}")
                    for j2 in range(2):
                        j = jh * 2 + j2
                        nc.tensor.matmul(
                            o_ps[:, j2 * 512 : (j2 + 1) * 512],
                            lhsT=uts[:, s4 * 128 : (s4 + 1) * 128],
                            rhs=B_sb[:, j * 512 : (j + 1) * 512],
                            start=True,
                            stop=True,
                        )
                    if copy_flip % 2 == 0:
                        nc.vector.tensor_copy(
                            out=o_sb[:, jh * 1024 : (jh + 1) * 1024], in_=o_ps
                        )
                    else:
                        nc.scalar.copy(
                            out=o_sb[:, jh * 1024 : (jh + 1) * 1024], in_=o_ps
                        )
                    copy_flip += 1
                nc.sync.dma_start(out=o_d[s * 128 : (s + 1) * 128, :], in_=o_sb)

    if split_waits:
        _split_matmul_waits(nc)
    return nc


def _split_matmul_waits(nc, max_waits=1):
    """Walrus codegen allows only one sync-wait slot on self-loading
    Matmult instructions. Move surplus waits onto a no-op EventSemaphore
    inserted immediately before, same engine — identical semantics."""
    import concourse.mybir as mybir

    n = 0
    for f in nc.m.functions:
        for blk in f.blocks:
            insts = blk.instructions
            new_list = []
            changed = False
            for inst in insts:
                si = inst.sync_info
                if (
                    type(inst).__name__ != "InstEventSemaphore"
                    and si is not None
                    and si.on_wait
                    and len(si.on_wait) > max_waits
                ):
                    surplus = list(si.on_wait[:-max_waits])
                    keep = list(si.on_wait[-max_waits:])
                    for i in range(0, len(surplus), 2):
                        n += 1
                        ev = mybir.InstEventSemaphore(
                            name=f"I-swsplit-{n}", ins=[], outs=[]
                        )
                        ev.engine = inst.engine
                        ev.sync_info = mybir.SyncInfo(
                            on_wait=surplus[i : i + 2], on_update=[]
                        )
                        new_list.append(ev)
                    inst.sync_info = mybir.SyncInfo(
                        on_wait=keep, on_update=list(si.on_update or [])
                    )
                    changed = True
                new_list.append(inst)
            if changed:
                blk.instructions = new_list
    return n


def _host_prep(h, p_L, A, B):
    """Shard tokens across cores; pre-transpose + pre-tile h; helpers."""
    import ml_dtypes

    BF16 = ml_dtypes.bfloat16

    # hT[core][p, g, k, t] = h[core][token g*GROUP+t, d = k*128+p]
    h5 = np.asarray(h, dtype=np.float32).reshape(N_CORES, N_GROUPS, GROUP, KD, 128)
    hT = np.ascontiguousarray(h5.transpose(0, 4, 1, 3, 2)).astype(BF16)
    hT = hT.reshape(N_CORES, 128, N_GROUPS * KD * GROUP)

    # permute p_L rows so partition p holds tokens {n*128+p}: row p*16+n
    p3 = np.asarray(p_L, dtype=np.float32).reshape(N_CORES, N_SUBTOT, 128, E)
    p_perm = np.ascontiguousarray(p3.transpose(0, 2, 1, 3)).reshape(
        N_CORES, T_CORE, E
    )

    # A_cat[d, c] = SCALING * A[e, r, d], pre-arranged [p, k*C + c]
    A_cat = (np.asarray(A, dtype=np.float32) * SCALING).transpose(2, 0, 1).reshape(D, C)
    A_arr = np.ascontiguousarray(
        A_cat.reshape(KD, 128, C).transpose(1, 0, 2).reshape(128, KD * C)
    ).astype(BF16)
    # B_cat[c, d] = B[e, d, r]
    B_cat = (
        np.asarray(B, dtype=np.float32).transpose(0, 2, 1).reshape(C, D).astype(BF16)
    )
    Mexp = np.zeros((E, C), dtype=np.float32)
    for e in range(E):
        Mexp[e, e * R : (e + 1) * R] = 1.0
    Ident = np.eye(128, dtype=np.float32)

    in_maps = []
    for i in range(N_CORES):
        in_maps.append(
            {
                "hT": hT[i],
                "p_perm": p_perm[i],
                "A_cat": A_arr,
                "B_cat": B_cat,
                "Mexp": Mexp,
                "Ident": Ident,
            }
        )
    return in_maps


def _get_nc():
    if "nc" not in _CACHE:
        _CACHE["nc"] = _build_nc()
    return _CACHE["nc"]


def kernel(h, p_L, A, B):
    from concourse.bass_utils import run_bass_kernel_spmd

    nc = _get_nc()
    in_maps = _host_prep(h, p_L, A, B)
    res = run_bass_kernel_spmd(nc, in_maps, core_ids=list(range(N_CORES)))
    out = np.concatenate(
        [np.asarray(res.results[i]["out"]) for i in range(N_CORES)], axis=0
    )
    return out.astype(np.float32).reshape(B_SZ, S_SZ, D)


# revision 9
# speedup vs baseline: 1.1575x; 1.1575x over previous
"""Trainium2 Bass kernel for nn_LoRAPool (MoE top-2 LoRA expert pool).

Math (reference):
    gates[t,e] = p_L[t,e] if e in top-2 of p_L[t,:] else 0
    hr[t,e,r]  = sum_d h[t,d] * A[e,r,d]
    out[t,d]   = sum_{e,r} hr[t,e,r] * 2.0 * gates[t,e] * B[e,d,r]

Folded into two dense matmuls over c = (e,r) in [0,128):
    A_cat[d,c] = 2.0 * A[e,r,d];  B_cat[c,d] = B[e,d,r]
    U^T[c,t]   = sum_d A_cat[d,c] hT[d,t]       (stage 1, PE, bf16)
    Us[c,t]    = U^T[c,t] * gates[t, c//16]     (gating, DVE, f32-exact gates)
    out[t,d]   = sum_c Us[c,t] B_cat[c,d]       (stage 2, PE, bf16)

Memory-bound: all large traffic (h in, out) is bf16 (tolerance 2e-2; bf16
end-to-end error is ~5e-3). h is pre-transposed AND pre-tiled on the host
([p, group, k, t] layout) so every device DMA has 8-16KB contiguous lines
and no on-device transposes are needed. Token groups of 512 pipeline:
group g's output store overlaps group g+1's input stream.

Sharding: tokens (4*4096 = 16384) split evenly across 8 cores; weights and
helper matrices replicated.
"""

import numpy as np

N_CORES = 8
B_SZ, S_SZ, D = 4, 4096, 2048
E, R, C = 8, 16, 128
T_FULL = B_SZ * S_SZ            # 16384 tokens
T_CORE = T_FULL // N_CORES      # 2048 tokens per core
GROUP = 512                     # token group (stage-1 PSUM bank width)
N_GROUPS = T_CORE // GROUP      # 4
N_SUBTOT = T_CORE // 128        # 16 sub-blocks of 128 tokens per core
SUB_PER_GROUP = GROUP // 128    # 4
KD = D // 128                   # 16 contraction chunks
KH = KD // 2                    # chunks per hT half-DMA
SCALING = 2.0

_CACHE = {}


def _build_nc(split_waits=True):
    import concourse.bass as bass
    import concourse.tile as tile
    import concourse.mybir as mybir
    from contextlib import ExitStack

    f32 = mybir.dt.float32
    f32r = mybir.dt.float32r
    bf16 = mybir.dt.bfloat16

    nc = bass.Bass()
    # hT[p, g*KD*GROUP + k*GROUP + t] = h[token g*GROUP+t, d = k*128+p]
    ht_d = nc.declare_dram_parameter(
        "hT", [128, N_GROUPS * KD * GROUP], bf16, isOutput=False
    )
    p_d = nc.declare_dram_parameter("p_perm", [T_CORE, E], f32, isOutput=False)
    a_d = nc.declare_dram_parameter("A_cat", [128, KD * C], bf16, isOutput=False)
    b_d = nc.declare_dram_parameter("B_cat", [C, D], bf16, isOutput=False)
    m_d = nc.declare_dram_parameter("Mexp", [E, C], f32, isOutput=False)
    i_d = nc.declare_dram_parameter("Ident", [128, 128], f32, isOutput=False)
    o_d = nc.declare_dram_parameter("out", [T_CORE, D], bf16, isOutput=True)

    AX = mybir.AxisListType
    OP = mybir.AluOpType

    with ExitStack() as ctx:
        tc = ctx.enter_context(tile.TileContext(nc))
        consts = ctx.enter_context(tc.tile_pool(name="consts", bufs=1))
        hpool = ctx.enter_context(tc.tile_pool(name="h", bufs=2 * N_GROUPS))
        gpool = ctx.enter_context(tc.tile_pool(name="gates", bufs=1))
        gtpool = ctx.enter_context(tc.tile_pool(name="gt", bufs=2))
        gsbpool = ctx.enter_context(tc.tile_pool(name="gsb", bufs=2))
        utspool = ctx.enter_context(tc.tile_pool(name="uts", bufs=2))
        opool = ctx.enter_context(tc.tile_pool(name="osb", bufs=3))
        ps_u = ctx.enter_context(tc.tile_pool(name="ps_u", bufs=1, space="PSUM"))
        ps_g = ctx.enter_context(tc.tile_pool(name="ps_g", bufs=1, space="PSUM"))
        ps_o = ctx.enter_context(tc.tile_pool(name="ps_o", bufs=3, space="PSUM"))

        # ---- constants + routing probs first (small, clears the queue) ----
        A_sb = consts.tile([128, KD * C], bf16)
        nc.sync.dma_start(out=A_sb, in_=a_d[:, :])
        p_sb = gpool.tile([128, N_SUBTOT, E], f32)
        nc.sync.dma_start(out=p_sb, in_=p_d.rearrange("(p n) e -> p n e", p=128))
        I_sb = consts.tile([128, 128], f32)
        nc.sync.dma_start(out=I_sb, in_=i_d[:, :])
        M_raw = consts.tile([E, C], f32)
        nc.sync.dma_start(out=M_raw, in_=m_d[:, :])
        M_sb = consts.tile([E, C], f32r)
        nc.vector.tensor_copy(out=M_sb, in_=M_raw)

        # ---- top-2 gates for the whole core: [128 tok, 16 sub, 8 exp] ----
        m1 = gpool.tile([128, N_SUBTOT, 1], f32)
        nc.vector.tensor_reduce(out=m1, in_=p_sb, axis=AX.X, op=OP.max)
        mlt = gpool.tile([128, N_SUBTOT, E], f32)
        nc.vector.tensor_tensor(
            out=mlt, in0=p_sb, in1=m1.broadcast_to([128, N_SUBTOT, E]), op=OP.is_lt
        )
        pm = gpool.tile([128, N_SUBTOT, E], f32)
        nc.vector.tensor_mul(pm, p_sb, mlt)
        m2 = gpool.tile([128, N_SUBTOT, 1], f32)
        nc.vector.tensor_reduce(out=m2, in_=pm, axis=AX.X, op=OP.max)
        ge2 = gpool.tile([128, N_SUBTOT, E], f32)
        nc.vector.tensor_tensor(
            out=ge2, in0=p_sb, in1=m2.broadcast_to([128, N_SUBTOT, E]), op=OP.is_ge
        )
        gts = gpool.tile([128, N_SUBTOT, E], f32)
        nc.vector.tensor_mul(gts, p_sb, ge2)

        B_issued = [False]

        def issue_ht(g):
            tiles = []
            for h2 in range(2):
                ht = hpool.tile([128, KH, GROUP], bf16, tag="h", name=f"ht{g}_{h2}")
                off = g * KD * GROUP + h2 * KH * GROUP
                nc.sync.dma_start(
                    out=ht,
                    in_=ht_d[:, off : off + KH * GROUP].rearrange(
                        "p (k t) -> p k t", k=KH
                    ),
                )
                tiles.append(ht)
            return tiles

        # issue ALL h loads before any output store enters the (in-order)
        # sync queue — otherwise stores head-of-line block later h streams
        ht_tiles = {0: issue_ht(0)}
        B_sb = consts.tile([C, D], bf16)
        nc.sync.dma_start(out=B_sb, in_=b_d[:, :])
        for g in range(1, N_GROUPS):
            ht_tiles[g] = issue_ht(g)

        def gates_for(g):
            # dense gate matrix G[c, t]: transpose + one-hot expand matmul
            gt_ps = ps_g.tile([128, GROUP], f32, tag="g", name=f"gt{g}")
            for s4 in range(SUB_PER_GROUP):
                s = g * SUB_PER_GROUP + s4
                nc.tensor.transpose(
                    out=gt_ps[:E, s4 * 128 : (s4 + 1) * 128],
                    in_=gts[:, s, :],
                    identity=I_sb,
                )
            gt_sb = gtpool.tile([E, GROUP], f32r, tag="gtsb", name=f"gtsb{g}")
            nc.vector.tensor_copy(out=gt_sb, in_=gt_ps[:E, :])
            G_ps = ps_g.tile([128, GROUP], f32, tag="g", name=f"G{g}")
            nc.tensor.matmul(G_ps, lhsT=M_sb, rhs=gt_sb, start=True, stop=True)
            G_sb = gsbpool.tile([128, GROUP], f32, tag="gsb", name=f"Gsb{g}")
            nc.scalar.copy(out=G_sb, in_=G_ps)
            return G_sb

        def stage1(g):
            U_ps = ps_u.tile([128, GROUP], f32, tag="u", name=f"U{g}")
            for k in range(KD):
                nc.tensor.matmul(
                    U_ps,
                    lhsT=A_sb[:, k * C : (k + 1) * C],
                    rhs=ht_tiles[g][k // KH][:, k % KH, :],
                    start=(k == 0),
                    stop=(k == KD - 1),
                )
            return U_ps

        copy_flip = [0]

        def stage2(g, U_ps, G_sb):
            uts = utspool.tile([128, GROUP], bf16, tag="uts", name=f"uts{g}")
            nc.vector.tensor_tensor(out=uts, in0=U_ps, in1=G_sb, op=OP.mult)
            for s4 in range(SUB_PER_GROUP):
                s = g * SUB_PER_GROUP + s4
                o_sb = opool.tile([128, D], bf16, tag="osb", name=f"osb{s}")
                for jh in range(2):
                    o_ps = ps_o.tile([128, 1024], f32, tag="o", name=f"o{s}_{jh}")
                    for j2 in range(2):
                        j = jh * 2 + j2
                        nc.tensor.matmul(
                            o_ps[:, j2 * 512 : (j2 + 1) * 512],
                            lhsT=uts[:, s4 * 128 : (s4 + 1) * 128],
                            rhs=B_sb[:, j * 512 : (j + 1) * 512],
                            start=True,
                            stop=True,
                        )
                    if copy_flip[0] % 2 == 0:
                        nc.vector.tensor_copy(
                            out=o_sb[:, jh * 1024 : (jh + 1) * 1024], in_=o_ps
                        )
                    else:
                        nc.scalar.copy(
                            out=o_sb[:, jh * 1024 : (jh + 1) * 1024], in_=o_ps
                        )
                    copy_flip[0] += 1
                nc.sync.dma_start(out=o_d[s * 128 : (s + 1) * 128, :], in_=o_sb)

        # PE order: gates g0 -> stage1 g0 -> gates g1..g3 (fills the wait
        # for group 1's h stream) -> stage2 g0 -> stage1 g1 -> stage2 g1 ...
        G_sbs = {0: gates_for(0)}
        U0 = stage1(0)
        for g in range(1, N_GROUPS):
            G_sbs[g] = gates_for(g)
        U_cur = U0
        for g in range(N_GROUPS):
            stage2(g, U_cur, G_sbs[g])
            if g + 1 < N_GROUPS:
                U_cur = stage1(g + 1)

    if split_waits:
        _split_matmul_waits(nc)
    return nc


def _split_matmul_waits(nc, max_waits=1):
    """Walrus codegen allows only one sync-wait slot on self-loading
    Matmult instructions. Move surplus waits onto a no-op EventSemaphore
    inserted immediately before, same engine — identical semantics."""
    import concourse.mybir as mybir

    n = 0
    for f in nc.m.functions:
        for blk in f.blocks:
            insts = blk.instructions
            new_list = []
            changed = False
            for inst in insts:
                si = inst.sync_info
                if (
                    type(inst).__name__ != "InstEventSemaphore"
                    and si is not None
                    and si.on_wait
                    and len(si.on_wait) > max_waits
                ):
                    surplus = list(si.on_wait[:-max_waits])
                    keep = list(si.on_wait[-max_waits:])
                    for i in range(0, len(surplus), 2):
                        n += 1
                        ev = mybir.InstEventSemaphore(
                            name=f"I-swsplit-{n}", ins=[], outs=[]
                        )
                        ev.engine = inst.engine
                        ev.sync_info = mybir.SyncInfo(
                            on_wait=surplus[i : i + 2], on_update=[]
                        )
                        new_list.append(ev)
                    inst.sync_info = mybir.SyncInfo(
                        on_wait=keep, on_update=list(si.on_update or [])
                    )
                    changed = True
                new_list.append(inst)
            if changed:
                blk.instructions = new_list
    return n


def _host_prep(h, p_L, A, B):
    """Shard tokens across cores; pre-transpose + pre-tile h; helpers."""
    import ml_dtypes

    BF16 = ml_dtypes.bfloat16

    # hT[core][p, g, k, t] = h[core][token g*GROUP+t, d = k*128+p]
    h5 = np.asarray(h, dtype=np.float32).reshape(N_CORES, N_GROUPS, GROUP, KD, 128)
    hT = np.ascontiguousarray(h5.transpose(0, 4, 1, 3, 2)).astype(BF16)
    hT = hT.reshape(N_CORES, 128, N_GROUPS * KD * GROUP)

    # permute p_L rows so partition p holds tokens {n*128+p}: row p*16+n
    p3 = np.asarray(p_L, dtype=np.float32).reshape(N_CORES, N_SUBTOT, 128, E)
    p_perm = np.ascontiguousarray(p3.transpose(0, 2, 1, 3)).reshape(
        N_CORES, T_CORE, E
    )

    # A_cat[d, c] = SCALING * A[e, r, d], pre-arranged [p, k*C + c]
    A_cat = (np.asarray(A, dtype=np.float32) * SCALING).transpose(2, 0, 1).reshape(D, C)
    A_arr = np.ascontiguousarray(
        A_cat.reshape(KD, 128, C).transpose(1, 0, 2).reshape(128, KD * C)
    ).astype(BF16)
    # B_cat[c, d] = B[e, d, r]
    B_cat = (
        np.asarray(B, dtype=np.float32).transpose(0, 2, 1).reshape(C, D).astype(BF16)
    )
    Mexp = np.zeros((E, C), dtype=np.float32)
    for e in range(E):
        Mexp[e, e * R : (e + 1) * R] = 1.0
    Ident = np.eye(128, dtype=np.float32)

    in_maps = []
    for i in range(N_CORES):
        in_maps.append(
            {
                "hT": hT[i],
                "p_perm": p_perm[i],
                "A_cat": A_arr,
                "B_cat": B_cat,
                "Mexp": Mexp,
                "Ident": Ident,
            }
        )
    return in_maps


def _get_nc():
    if "nc" not in _CACHE:
        _CACHE["nc"] = _build_nc()
    return _CACHE["nc"]


def kernel(h, p_L, A, B):
    from concourse.bass_utils import run_bass_kernel_spmd

    nc = _get_nc()
    in_maps = _host_prep(h, p_L, A, B)
    res = run_bass_kernel_spmd(nc, in_maps, core_ids=list(range(N_CORES)))
    out = np.concatenate(
        [np.asarray(res.results[i]["out"]) for i in range(N_CORES)], axis=0
    )
    return out.astype(np.float32).reshape(B_SZ, S_SZ, D)


# revision 12
# speedup vs baseline: 1.2265x; 1.0595x over previous
"""Trainium2 Bass kernel for nn_LoRAPool (MoE top-2 LoRA expert pool).

Math (reference):
    gates[t,e] = p_L[t,e] if e in top-2 of p_L[t,:] else 0
    hr[t,e,r]  = sum_d h[t,d] * A[e,r,d]
    out[t,d]   = sum_{e,r} hr[t,e,r] * 2.0 * gates[t,e] * B[e,d,r]

Folded into two dense matmuls over c = (e,r) in [0,128):
    A_cat[d,c] = 2.0 * A[e,r,d];  B_cat[c,d] = B[e,d,r]
    U^T[c,t]   = sum_d A_cat[d,c] hT[d,t]       (stage 1, PE, bf16)
    Us[c,t]    = U^T[c,t] * gates[t, c//16]     (gating, DVE, f32-exact gates)
    out[t,d]   = sum_c Us[c,t] B_cat[c,d]       (stage 2, PE, bf16)

Memory-bound: all large traffic (h in, out) is bf16 (tolerance 2e-2; bf16
end-to-end error is ~5e-3). h is pre-transposed AND pre-tiled on the host
([p, group, k, t] layout) so every device DMA has 8-16KB contiguous lines
and no on-device transposes are needed. Token groups of 512 pipeline:
group g's output store overlaps group g+1's input stream.

Sharding: tokens (4*4096 = 16384) split evenly across 8 cores; weights and
helper matrices replicated.
"""

import numpy as np

N_CORES = 8
B_SZ, S_SZ, D = 4, 4096, 2048
E, R, C = 8, 16, 128
T_FULL = B_SZ * S_SZ            # 16384 tokens
T_CORE = T_FULL // N_CORES      # 2048 tokens per core
GROUP = 512                     # token group (stage-1 PSUM bank width)
N_GROUPS = T_CORE // GROUP      # 4
N_SUBTOT = T_CORE // 128        # 16 sub-blocks of 128 tokens per core
SUB_PER_GROUP = GROUP // 128    # 4
KD = D // 128                   # 16 contraction chunks
KH = KD // 2                    # chunks per hT half-DMA
SCALING = 2.0

_CACHE = {}


def _build_nc(split_waits=True):
    import concourse.bass as bass
    import concourse.tile as tile
    import concourse.mybir as mybir
    from contextlib import ExitStack

    f32 = mybir.dt.float32
    f32r = mybir.dt.float32r
    bf16 = mybir.dt.bfloat16

    nc = bass.Bass()
    # hT[p, g*KD*GROUP + k*GROUP + t] = h[token g*GROUP+t, d = k*128+p]
    ht_d = nc.declare_dram_parameter(
        "hT", [128, N_GROUPS * KD * GROUP], bf16, isOutput=False
    )
    p_d = nc.declare_dram_parameter("p_perm", [T_CORE, E], f32, isOutput=False)
    a_d = nc.declare_dram_parameter("A_cat", [128, KD * C], bf16, isOutput=False)
    b_d = nc.declare_dram_parameter("B_cat", [C, D], bf16, isOutput=False)
    m_d = nc.declare_dram_parameter("Mexp", [E, C], f32, isOutput=False)
    i_d = nc.declare_dram_parameter("Ident", [128, 128], f32, isOutput=False)
    o_d = nc.declare_dram_parameter("out", [T_CORE, D], bf16, isOutput=True)

    AX = mybir.AxisListType
    OP = mybir.AluOpType

    with ExitStack() as ctx:
        tc = ctx.enter_context(tile.TileContext(nc))
        consts = ctx.enter_context(tc.tile_pool(name="consts", bufs=1))
        hpool = ctx.enter_context(tc.tile_pool(name="h", bufs=2 * N_GROUPS))
        gpool = ctx.enter_context(tc.tile_pool(name="gates", bufs=1))
        gtpool = ctx.enter_context(tc.tile_pool(name="gt", bufs=2))
        gsbpool = ctx.enter_context(tc.tile_pool(name="gsb", bufs=2))
        utspool = ctx.enter_context(tc.tile_pool(name="uts", bufs=2))
        opool = ctx.enter_context(tc.tile_pool(name="osb", bufs=3))
        ps_u = ctx.enter_context(tc.tile_pool(name="ps_u", bufs=2, space="PSUM"))
        # gates (early) and stage-2 out tiles (late) share 3 two-bank slots
        ps_o = ctx.enter_context(tc.tile_pool(name="ps_o", bufs=3, space="PSUM"))

        # ---- constants + routing probs first (small, clears the queue) ----
        A_sb = consts.tile([128, KD * C], bf16)
        nc.sync.dma_start(out=A_sb, in_=a_d[:, :])
        p_sb = gpool.tile([128, N_SUBTOT, E], f32)
        nc.sync.dma_start(out=p_sb, in_=p_d.rearrange("(p n) e -> p n e", p=128))
        I_sb = consts.tile([128, 128], f32)
        nc.sync.dma_start(out=I_sb, in_=i_d[:, :])
        M_raw = consts.tile([E, C], f32)
        nc.sync.dma_start(out=M_raw, in_=m_d[:, :])
        M_sb = consts.tile([E, C], f32r)
        nc.vector.tensor_copy(out=M_sb, in_=M_raw)

        # ---- top-2 gates for the whole core: [128 tok, 16 sub, 8 exp] ----
        m1 = gpool.tile([128, N_SUBTOT, 1], f32)
        nc.vector.tensor_reduce(out=m1, in_=p_sb, axis=AX.X, op=OP.max)
        mlt = gpool.tile([128, N_SUBTOT, E], f32)
        nc.vector.tensor_tensor(
            out=mlt, in0=p_sb, in1=m1.broadcast_to([128, N_SUBTOT, E]), op=OP.is_lt
        )
        pm = gpool.tile([128, N_SUBTOT, E], f32)
        nc.vector.tensor_mul(pm, p_sb, mlt)
        m2 = gpool.tile([128, N_SUBTOT, 1], f32)
        nc.vector.tensor_reduce(out=m2, in_=pm, axis=AX.X, op=OP.max)
        ge2 = gpool.tile([128, N_SUBTOT, E], f32)
        nc.vector.tensor_tensor(
            out=ge2, in0=p_sb, in1=m2.broadcast_to([128, N_SUBTOT, E]), op=OP.is_ge
        )
        gts = gpool.tile([128, N_SUBTOT, E], f32)
        nc.vector.tensor_mul(gts, p_sb, ge2)

        B_issued = [False]

        def issue_ht(g):
            tiles = []
            for h2 in range(2):
                ht = hpool.tile([128, KH, GROUP], bf16, tag="h", name=f"ht{g}_{h2}")
                off = g * KD * GROUP + h2 * KH * GROUP
                nc.sync.dma_start(
                    out=ht,
                    in_=ht_d[:, off : off + KH * GROUP].rearrange(
                        "p (k t) -> p k t", k=KH
                    ),
                )
                tiles.append(ht)
            return tiles

        # issue ALL h loads before any output store enters the (in-order)
        # sync queue — otherwise stores head-of-line block later h streams
        ht_tiles = {0: issue_ht(0)}
        B_sb = consts.tile([C, D], bf16)
        nc.sync.dma_start(out=B_sb, in_=b_d[:, :])
        for g in range(1, N_GROUPS):
            ht_tiles[g] = issue_ht(g)

        def gates_for(g):
            # dense gate matrix G[c, t]: transpose + one-hot expand matmul
            # ([128,1024]-shaped allocs keep the shared pool's slots uniform)
            gt_ps = ps_o.tile([128, 1024], f32, tag="o", name=f"gt{g}")
            for s4 in range(SUB_PER_GROUP):
                s = g * SUB_PER_GROUP + s4
                nc.tensor.transpose(
                    out=gt_ps[:E, s4 * 128 : (s4 + 1) * 128],
                    in_=gts[:, s, :],
                    identity=I_sb,
                )
            gt_sb = gtpool.tile([E, GROUP], f32r, tag="gtsb", name=f"gtsb{g}")
            nc.vector.tensor_copy(out=gt_sb, in_=gt_ps[:E, :GROUP])
            G_ps = ps_o.tile([128, 1024], f32, tag="o", name=f"G{g}")
            nc.tensor.matmul(
                G_ps[:, :GROUP], lhsT=M_sb, rhs=gt_sb, start=True, stop=True
            )
            G_sb = gsbpool.tile([128, GROUP], f32, tag="gsb", name=f"Gsb{g}")
            nc.scalar.copy(out=G_sb, in_=G_ps[:, :GROUP])
            return G_sb

        def stage1(g):
            U_ps = ps_u.tile([128, GROUP], f32, tag="u", name=f"U{g}")
            for k in range(KD):
                nc.tensor.matmul(
                    U_ps,
                    lhsT=A_sb[:, k * C : (k + 1) * C],
                    rhs=ht_tiles[g][k // KH][:, k % KH, :],
                    start=(k == 0),
                    stop=(k == KD - 1),
                )
            return U_ps

        copy_flip = [0]

        def stage2(g, U_ps, G_sb):
            uts = utspool.tile([128, GROUP], bf16, tag="uts", name=f"uts{g}")
            nc.vector.tensor_tensor(out=uts, in0=U_ps, in1=G_sb, op=OP.mult)
            for s4 in range(SUB_PER_GROUP):
                s = g * SUB_PER_GROUP + s4
                o_sb = opool.tile([128, D], bf16, tag="osb", name=f"osb{s}")
                for jh in range(2):
                    o_ps = ps_o.tile([128, 1024], f32, tag="o", name=f"o{s}_{jh}")
                    for j2 in range(2):
                        j = jh * 2 + j2
                        nc.tensor.matmul(
                            o_ps[:, j2 * 512 : (j2 + 1) * 512],
                            lhsT=uts[:, s4 * 128 : (s4 + 1) * 128],
                            rhs=B_sb[:, j * 512 : (j + 1) * 512],
                            start=True,
                            stop=True,
                        )
                    if copy_flip[0] % 2 == 0:
                        nc.vector.tensor_copy(
                            out=o_sb[:, jh * 1024 : (jh + 1) * 1024], in_=o_ps
                        )
                    else:
                        nc.scalar.copy(
                            out=o_sb[:, jh * 1024 : (jh + 1) * 1024], in_=o_ps
                        )
                    copy_flip[0] += 1
                nc.sync.dma_start(out=o_d[s * 128 : (s + 1) * 128, :], in_=o_sb)

        # PE order: gates g0 -> stage1 g0 -> gates g1..g3 (fills the wait
        # for group 1's h stream) -> stage2 g0 -> stage1 g1 -> stage2 g1 ...
        G_sbs = {0: gates_for(0)}
        U0 = stage1(0)
        for g in range(1, N_GROUPS):
            G_sbs[g] = gates_for(g)
        U_cur = U0
        for g in range(N_GROUPS):
            stage2(g, U_cur, G_sbs[g])
            if g + 1 < N_GROUPS:
                U_cur = stage1(g + 1)

    if split_waits:
        _split_matmul_waits(nc)
    return nc


def _split_matmul_waits(nc, max_waits=1):
    """Walrus codegen allows only one sync-wait slot on self-loading
    Matmult instructions. Move surplus waits onto a no-op EventSemaphore
    inserted immediately before, same engine — identical semantics."""
    import concourse.mybir as mybir

    n = 0
    for f in nc.m.functions:
        for blk in f.blocks:
            insts = blk.instructions
            new_list = []
            changed = False
            for inst in insts:
                si = inst.sync_info
                if (
                    type(inst).__name__ != "InstEventSemaphore"
                    and si is not None
                    and si.on_wait
                    and len(si.on_wait) > max_waits
                ):
                    surplus = list(si.on_wait[:-max_waits])
                    keep = list(si.on_wait[-max_waits:])
                    for i in range(0, len(surplus), 2):
                        n += 1
                        ev = mybir.InstEventSemaphore(
                            name=f"I-swsplit-{n}", ins=[], outs=[]
                        )
                        ev.engine = inst.engine
                        ev.sync_info = mybir.SyncInfo(
                            on_wait=surplus[i : i + 2], on_update=[]
                        )
                        new_list.append(ev)
                    inst.sync_info = mybir.SyncInfo(
                        on_wait=keep, on_update=list(si.on_update or [])
                    )
                    changed = True
                new_list.append(inst)
            if changed:
                blk.instructions = new_list
    return n


def _host_prep(h, p_L, A, B):
    """Shard tokens across cores; pre-transpose + pre-tile h; helpers."""
    import ml_dtypes

    BF16 = ml_dtypes.bfloat16

    # hT[core][p, g, k, t] = h[core][token g*GROUP+t, d = k*128+p]
    h5 = np.asarray(h, dtype=np.float32).reshape(N_CORES, N_GROUPS, GROUP, KD, 128)
    hT = np.ascontiguousarray(h5.transpose(0, 4, 1, 3, 2)).astype(BF16)
    hT = hT.reshape(N_CORES, 128, N_GROUPS * KD * GROUP)

    # permute p_L rows so partition p holds tokens {n*128+p}: row p*16+n
    p3 = np.asarray(p_L, dtype=np.float32).reshape(N_CORES, N_SUBTOT, 128, E)
    p_perm = np.ascontiguousarray(p3.transpose(0, 2, 1, 3)).reshape(
        N_CORES, T_CORE, E
    )

    # A_cat[d, c] = SCALING * A[e, r, d], pre-arranged [p, k*C + c]
    A_cat = (np.asarray(A, dtype=np.float32) * SCALING).transpose(2, 0, 1).reshape(D, C)
    A_arr = np.ascontiguousarray(
        A_cat.reshape(KD, 128, C).transpose(1, 0, 2).reshape(128, KD * C)
    ).astype(BF16)
    # B_cat[c, d] = B[e, d, r]
    B_cat = (
        np.asarray(B, dtype=np.float32).transpose(0, 2, 1).reshape(C, D).astype(BF16)
    )
    Mexp = np.zeros((E, C), dtype=np.float32)
    for e in range(E):
        Mexp[e, e * R : (e + 1) * R] = 1.0
    Ident = np.eye(128, dtype=np.float32)

    in_maps = []
    for i in range(N_CORES):
        in_maps.append(
            {
                "hT": hT[i],
                "p_perm": p_perm[i],
                "A_cat": A_arr,
                "B_cat": B_cat,
                "Mexp": Mexp,
                "Ident": Ident,
            }
        )
    return in_maps


def _get_nc():
    if "nc" not in _CACHE:
        _CACHE["nc"] = _build_nc()
    return _CACHE["nc"]


def kernel(h, p_L, A, B):
    from concourse.bass_utils import run_bass_kernel_spmd

    nc = _get_nc()
    in_maps = _host_prep(h, p_L, A, B)
    res = run_bass_kernel_spmd(nc, in_maps, core_ids=list(range(N_CORES)))
    out = np.concatenate(
        [np.asarray(res.results[i]["out"]) for i in range(N_CORES)], axis=0
    )
    return out.astype(np.float32).reshape(B_SZ, S_SZ, D)


# revision 13
# speedup vs baseline: 1.2409x; 1.0118x over previous
"""Trainium2 Bass kernel for nn_LoRAPool (MoE top-2 LoRA expert pool).

Math (reference):
    gates[t,e] = p_L[t,e] if e in top-2 of p_L[t,:] else 0
    hr[t,e,r]  = sum_d h[t,d] * A[e,r,d]
    out[t,d]   = sum_{e,r} hr[t,e,r] * 2.0 * gates[t,e] * B[e,d,r]

Folded into two dense matmuls over c = (e,r) in [0,128):
    A_cat[d,c] = 2.0 * A[e,r,d];  B_cat[c,d] = B[e,d,r]
    U^T[c,t]   = sum_d A_cat[d,c] hT[d,t]       (stage 1, PE, bf16)
    Us[c,t]    = U^T[c,t] * gates[t, c//16]     (gating, DVE, f32-exact gates)
    out[t,d]   = sum_c Us[c,t] B_cat[c,d]       (stage 2, PE, bf16)

Memory-bound: all large traffic (h in, out) is bf16 (tolerance 2e-2; bf16
end-to-end error is ~5e-3). h is pre-transposed AND pre-tiled on the host
([p, group, k, t] layout) so every device DMA has 8-16KB contiguous lines
and no on-device transposes are needed. Token groups of 512 pipeline:
group g's output store overlaps group g+1's input stream.

Sharding: tokens (4*4096 = 16384) split evenly across 8 cores; weights and
helper matrices replicated.
"""

import numpy as np

N_CORES = 8
B_SZ, S_SZ, D = 4, 4096, 2048
E, R, C = 8, 16, 128
T_FULL = B_SZ * S_SZ            # 16384 tokens
T_CORE = T_FULL // N_CORES      # 2048 tokens per core
GROUP = 512                     # token group (stage-1 PSUM bank width)
N_GROUPS = T_CORE // GROUP      # 4
N_SUBTOT = T_CORE // 128        # 16 sub-blocks of 128 tokens per core
SUB_PER_GROUP = GROUP // 128    # 4
KD = D // 128                   # 16 contraction chunks
KH = KD // 2                    # chunks per hT half-DMA
SCALING = 2.0

_CACHE = {}


def _build_nc(split_waits=True):
    import concourse.bass as bass
    import concourse.tile as tile
    import concourse.mybir as mybir
    from contextlib import ExitStack

    f32 = mybir.dt.float32
    f32r = mybir.dt.float32r
    bf16 = mybir.dt.bfloat16

    nc = bass.Bass()
    # hT[p, g*KD*GROUP + k*GROUP + t] = h[token g*GROUP+t, d = k*128+p]
    ht_d = nc.declare_dram_parameter(
        "hT", [128, N_GROUPS * KD * GROUP], bf16, isOutput=False
    )
    p_d = nc.declare_dram_parameter("p_perm", [T_CORE, E], f32, isOutput=False)
    a_d = nc.declare_dram_parameter("A_cat", [128, KD * C], bf16, isOutput=False)
    b_d = nc.declare_dram_parameter("B_cat", [C, D], bf16, isOutput=False)
    m_d = nc.declare_dram_parameter("Mexp", [E, C], f32, isOutput=False)
    i_d = nc.declare_dram_parameter("Ident", [128, 128], f32, isOutput=False)
    o_d = nc.declare_dram_parameter("out", [T_CORE, D], bf16, isOutput=True)

    AX = mybir.AxisListType
    OP = mybir.AluOpType

    with ExitStack() as ctx:
        tc = ctx.enter_context(tile.TileContext(nc))
        consts = ctx.enter_context(tc.tile_pool(name="consts", bufs=1))
        hpool = ctx.enter_context(tc.tile_pool(name="h", bufs=2 * N_GROUPS))
        gpool = ctx.enter_context(tc.tile_pool(name="gates", bufs=1))
        gtpool = ctx.enter_context(tc.tile_pool(name="gt", bufs=2))
        gsbpool = ctx.enter_context(tc.tile_pool(name="gsb", bufs=2))
        utspool = ctx.enter_context(tc.tile_pool(name="uts", bufs=2))
        opool = ctx.enter_context(tc.tile_pool(name="osb", bufs=3))
        ps_u = ctx.enter_context(tc.tile_pool(name="ps_u", bufs=2, space="PSUM"))
        # gates (early) and stage-2 out tiles (late) share 3 two-bank slots
        ps_o = ctx.enter_context(tc.tile_pool(name="ps_o", bufs=3, space="PSUM"))

        # ---- constants + routing probs first (small, clears the queue) ----
        A_sb = consts.tile([128, KD * C], bf16)
        nc.sync.dma_start(out=A_sb, in_=a_d[:, :])
        p_sb = gpool.tile([128, N_SUBTOT, E], f32)
        nc.sync.dma_start(out=p_sb, in_=p_d.rearrange("(p n) e -> p n e", p=128))
        I_sb = consts.tile([128, 128], f32)
        nc.sync.dma_start(out=I_sb, in_=i_d[:, :])
        M_raw = consts.tile([E, C], f32)
        nc.sync.dma_start(out=M_raw, in_=m_d[:, :])
        M_sb = consts.tile([E, C], f32r)
        nc.vector.tensor_copy(out=M_sb, in_=M_raw)

        # ---- top-2 gates for the whole core: [128 tok, 16 sub, 8 exp] ----
        m1 = gpool.tile([128, N_SUBTOT, 1], f32)
        nc.vector.tensor_reduce(out=m1, in_=p_sb, axis=AX.X, op=OP.max)
        mlt = gpool.tile([128, N_SUBTOT, E], f32)
        nc.vector.tensor_tensor(
            out=mlt, in0=p_sb, in1=m1.broadcast_to([128, N_SUBTOT, E]), op=OP.is_lt
        )
        pm = gpool.tile([128, N_SUBTOT, E], f32)
        nc.vector.tensor_mul(pm, p_sb, mlt)
        m2 = gpool.tile([128, N_SUBTOT, 1], f32)
        nc.vector.tensor_reduce(out=m2, in_=pm, axis=AX.X, op=OP.max)
        ge2 = gpool.tile([128, N_SUBTOT, E], f32)
        nc.vector.tensor_tensor(
            out=ge2, in0=p_sb, in1=m2.broadcast_to([128, N_SUBTOT, E]), op=OP.is_ge
        )
        gts = gpool.tile([128, N_SUBTOT, E], f32)
        nc.vector.tensor_mul(gts, p_sb, ge2)

        B_issued = [False]

        def issue_ht(g):
            tiles = []
            for h2 in range(2):
                ht = hpool.tile([128, KH, GROUP], bf16, tag="h", name=f"ht{g}_{h2}")
                off = g * KD * GROUP + h2 * KH * GROUP
                nc.sync.dma_start(
                    out=ht,
                    in_=ht_d[:, off : off + KH * GROUP].rearrange(
                        "p (k t) -> p k t", k=KH
                    ),
                )
                tiles.append(ht)
            return tiles

        # issue ALL h loads before any output store enters the (in-order)
        # sync queue — otherwise stores head-of-line block later h streams
        ht_tiles = {0: issue_ht(0)}
        B_sb = consts.tile([C, D], bf16)
        nc.sync.dma_start(out=B_sb, in_=b_d[:, :])
        for g in range(1, N_GROUPS):
            ht_tiles[g] = issue_ht(g)

        def gates_for(g):
            # dense gate matrix G[c, t]: transpose + one-hot expand matmul
            # ([128,1024]-shaped allocs keep the shared pool's slots uniform)
            gt_ps = ps_o.tile([128, 1024], f32, tag="o", name=f"gt{g}")
            for s4 in range(SUB_PER_GROUP):
                s = g * SUB_PER_GROUP + s4
                nc.tensor.transpose(
                    out=gt_ps[:E, s4 * 128 : (s4 + 1) * 128],
                    in_=gts[:, s, :],
                    identity=I_sb,
                )
            gt_sb = gtpool.tile([E, GROUP], f32r, tag="gtsb", name=f"gtsb{g}")
            nc.vector.tensor_copy(out=gt_sb, in_=gt_ps[:E, :GROUP])
            G_ps = ps_o.tile([128, 1024], f32, tag="o", name=f"G{g}")
            nc.tensor.matmul(
                G_ps[:, :GROUP], lhsT=M_sb, rhs=gt_sb, start=True, stop=True
            )
            G_sb = gsbpool.tile([128, GROUP], f32, tag="gsb", name=f"Gsb{g}")
            nc.scalar.copy(out=G_sb, in_=G_ps[:, :GROUP])
            return G_sb

        def stage1(g):
            U_ps = ps_u.tile([128, GROUP], f32, tag="u", name=f"U{g}")
            for k in range(KD):
                nc.tensor.matmul(
                    U_ps,
                    lhsT=A_sb[:, k * C : (k + 1) * C],
                    rhs=ht_tiles[g][k // KH][:, k % KH, :],
                    start=(k == 0),
                    stop=(k == KD - 1),
                )
            return U_ps

        copy_flip = [0]

        def stage2(g, U_ps, G_sb):
            uts = utspool.tile([128, GROUP], bf16, tag="uts", name=f"uts{g}")
            nc.vector.tensor_tensor(out=uts, in0=U_ps, in1=G_sb, op=OP.mult)
            for s4 in range(SUB_PER_GROUP):
                s = g * SUB_PER_GROUP + s4
                o_sb = opool.tile([128, D], bf16, tag="osb", name=f"osb{s}")
                for jh in range(2):
                    o_ps = ps_o.tile([128, 1024], f32, tag="o", name=f"o{s}_{jh}")
                    for j2 in range(2):
                        j = jh * 2 + j2
                        nc.tensor.matmul(
                            o_ps[:, j2 * 512 : (j2 + 1) * 512],
                            lhsT=uts[:, s4 * 128 : (s4 + 1) * 128],
                            rhs=B_sb[:, j * 512 : (j + 1) * 512],
                            start=True,
                            stop=True,
                        )
                    if copy_flip[0] % 2 == 0:
                        nc.vector.tensor_copy(
                            out=o_sb[:, jh * 1024 : (jh + 1) * 1024], in_=o_ps
                        )
                    else:
                        nc.scalar.copy(
                            out=o_sb[:, jh * 1024 : (jh + 1) * 1024], in_=o_ps
                        )
                    copy_flip[0] += 1
                nc.sync.dma_start(out=o_d[s * 128 : (s + 1) * 128, :], in_=o_sb)

        # PE order: gates g0 -> stage1 g0 -> gates g1..g3 (fills the wait
        # for group 1's h stream) -> stage2 g0 -> stage1 g1 -> stage2 g1 ...
        # Monotone logical waits stop the scheduler from hoisting group g+1
        # work above group g's store pipeline (which starves the out DMAs).
        G_sbs = {0: gates_for(0)}
        U0 = stage1(0)
        for g in range(1, N_GROUPS):
            G_sbs[g] = gates_for(g)
        U_cur = U0
        for g in range(N_GROUPS):
            tc.tile_set_cur_wait(g + 1)
            stage2(g, U_cur, G_sbs[g])
            if g + 1 < N_GROUPS:
                U_cur = stage1(g + 1)

    if split_waits:
        _split_matmul_waits(nc)
    return nc


def _split_matmul_waits(nc, max_waits=1):
    """Walrus codegen allows only one sync-wait slot on self-loading
    Matmult instructions. Move surplus waits onto a no-op EventSemaphore
    inserted immediately before, same engine — identical semantics."""
    import concourse.mybir as mybir

    n = 0
    for f in nc.m.functions:
        for blk in f.blocks:
            insts = blk.instructions
            new_list = []
            changed = False
            for inst in insts:
                si = inst.sync_info
                if (
                    type(inst).__name__ != "InstEventSemaphore"
                    and si is not None
                    and si.on_wait
                    and len(si.on_wait) > max_waits
                ):
                    surplus = list(si.on_wait[:-max_waits])
                    keep = list(si.on_wait[-max_waits:])
                    for i in range(0, len(surplus), 2):
                        n += 1
                        ev = mybir.InstEventSemaphore(
                            name=f"I-swsplit-{n}", ins=[], outs=[]
                        )
                        ev.engine = inst.engine
                        ev.sync_info = mybir.SyncInfo(
                            on_wait=surplus[i : i + 2], on_update=[]
                        )
                        new_list.append(ev)
                    inst.sync_info = mybir.SyncInfo(
                        on_wait=keep, on_update=list(si.on_update or [])
                    )
                    changed = True
                new_list.append(inst)
            if changed:
                blk.instructions = new_list
    return n


def _host_prep(h, p_L, A, B):
    """Shard tokens across cores; pre-transpose + pre-tile h; helpers."""
    import ml_dtypes

    BF16 = ml_dtypes.bfloat16

    # hT[core][p, g, k, t] = h[core][token g*GROUP+t, d = k*128+p]
    h5 = np.asarray(h, dtype=np.float32).reshape(N_CORES, N_GROUPS, GROUP, KD, 128)
    hT = np.ascontiguousarray(h5.transpose(0, 4, 1, 3, 2)).astype(BF16)
    hT = hT.reshape(N_CORES, 128, N_GROUPS * KD * GROUP)

    # permute p_L rows so partition p holds tokens {n*128+p}: row p*16+n
    p3 = np.asarray(p_L, dtype=np.float32).reshape(N_CORES, N_SUBTOT, 128, E)
    p_perm = np.ascontiguousarray(p3.transpose(0, 2, 1, 3)).reshape(
        N_CORES, T_CORE, E
    )

    # A_cat[d, c] = SCALING * A[e, r, d], pre-arranged [p, k*C + c]
    A_cat = (np.asarray(A, dtype=np.float32) * SCALING).transpose(2, 0, 1).reshape(D, C)
    A_arr = np.ascontiguousarray(
        A_cat.reshape(KD, 128, C).transpose(1, 0, 2).reshape(128, KD * C)
    ).astype(BF16)
    # B_cat[c, d] = B[e, d, r]
    B_cat = (
        np.asarray(B, dtype=np.float32).transpose(0, 2, 1).reshape(C, D).astype(BF16)
    )
    Mexp = np.zeros((E, C), dtype=np.float32)
    for e in range(E):
        Mexp[e, e * R : (e + 1) * R] = 1.0
    Ident = np.eye(128, dtype=np.float32)

    in_maps = []
    for i in range(N_CORES):
        in_maps.append(
            {
                "hT": hT[i],
                "p_perm": p_perm[i],
                "A_cat": A_arr,
                "B_cat": B_cat,
                "Mexp": Mexp,
                "Ident": Ident,
            }
        )
    return in_maps


def _get_nc():
    if "nc" not in _CACHE:
        _CACHE["nc"] = _build_nc()
    return _CACHE["nc"]


def kernel(h, p_L, A, B):
    from concourse.bass_utils import run_bass_kernel_spmd

    nc = _get_nc()
    in_maps = _host_prep(h, p_L, A, B)
    res = run_bass_kernel_spmd(nc, in_maps, core_ids=list(range(N_CORES)))
    out = np.concatenate(
        [np.asarray(res.results[i]["out"]) for i in range(N_CORES)], axis=0
    )
    return out.astype(np.float32).reshape(B_SZ, S_SZ, D)


# revision 14
# speedup vs baseline: 1.2518x; 1.0088x over previous
"""Trainium2 Bass kernel for nn_LoRAPool (MoE top-2 LoRA expert pool).

Math (reference):
    gates[t,e] = p_L[t,e] if e in top-2 of p_L[t,:] else 0
    hr[t,e,r]  = sum_d h[t,d] * A[e,r,d]
    out[t,d]   = sum_{e,r} hr[t,e,r] * 2.0 * gates[t,e] * B[e,d,r]

Folded into two dense matmuls over c = (e,r) in [0,128):
    A_cat[d,c] = 2.0 * A[e,r,d];  B_cat[c,d] = B[e,d,r]
    U^T[c,t]   = sum_d A_cat[d,c] hT[d,t]       (stage 1, PE, bf16)
    Us[c,t]    = U^T[c,t] * gates[t, c//16]     (gating, DVE, f32-exact gates)
    out[t,d]   = sum_c Us[c,t] B_cat[c,d]       (stage 2, PE, bf16)

Memory-bound: all large traffic (h in, out) is bf16 (tolerance 2e-2; bf16
end-to-end error is ~5e-3). h is pre-transposed AND pre-tiled on the host
([p, group, k, t] layout) so every device DMA has 8-16KB contiguous lines
and no on-device transposes are needed. Token groups of 512 pipeline:
group g's output store overlaps group g+1's input stream.

Sharding: tokens (4*4096 = 16384) split evenly across 8 cores; weights and
helper matrices replicated.
"""

import numpy as np

N_CORES = 8
B_SZ, S_SZ, D = 4, 4096, 2048
E, R, C = 8, 16, 128
T_FULL = B_SZ * S_SZ            # 16384 tokens
T_CORE = T_FULL // N_CORES      # 2048 tokens per core
GROUP = 512                     # token group (stage-1 PSUM bank width)
N_GROUPS = T_CORE // GROUP      # 4
N_SUBTOT = T_CORE // 128        # 16 sub-blocks of 128 tokens per core
SUB_PER_GROUP = GROUP // 128    # 4
KD = D // 128                   # 16 contraction chunks
KH = KD // 2                    # chunks per hT half-DMA
SCALING = 2.0

_CACHE = {}


def _build_nc(split_waits=True):
    import concourse.bass as bass
    import concourse.tile as tile
    import concourse.mybir as mybir
    from contextlib import ExitStack

    f32 = mybir.dt.float32
    f32r = mybir.dt.float32r
    bf16 = mybir.dt.bfloat16

    nc = bass.Bass()
    # hT[p, g*KD*GROUP + k*GROUP + t] = h[token g*GROUP+t, d = k*128+p]
    ht_d = nc.declare_dram_parameter(
        "hT", [128, N_GROUPS * KD * GROUP], bf16, isOutput=False
    )
    p_d = nc.declare_dram_parameter("p_perm", [T_CORE, E], f32, isOutput=False)
    a_d = nc.declare_dram_parameter("A_cat", [128, KD * C], bf16, isOutput=False)
    b_d = nc.declare_dram_parameter("B_cat", [C, D], bf16, isOutput=False)
    m_d = nc.declare_dram_parameter("Mexp", [E, C], f32, isOutput=False)
    i_d = nc.declare_dram_parameter("Ident", [128, 128], f32, isOutput=False)
    o_d = nc.declare_dram_parameter("out", [T_CORE, D], bf16, isOutput=True)

    AX = mybir.AxisListType
    OP = mybir.AluOpType

    with ExitStack() as ctx:
        tc = ctx.enter_context(tile.TileContext(nc))
        consts = ctx.enter_context(tc.tile_pool(name="consts", bufs=1))
        hpool = ctx.enter_context(tc.tile_pool(name="h", bufs=2 * N_GROUPS))
        gpool = ctx.enter_context(tc.tile_pool(name="gates", bufs=1))
        gtpool = ctx.enter_context(tc.tile_pool(name="gt", bufs=2))
        gsbpool = ctx.enter_context(tc.tile_pool(name="gsb", bufs=2))
        utspool = ctx.enter_context(tc.tile_pool(name="uts", bufs=2))
        opool = ctx.enter_context(tc.tile_pool(name="osb", bufs=3))
        ps_u = ctx.enter_context(tc.tile_pool(name="ps_u", bufs=2, space="PSUM"))
        # gates (early) and stage-2 out tiles (late) share 3 two-bank slots
        ps_o = ctx.enter_context(tc.tile_pool(name="ps_o", bufs=3, space="PSUM"))

        # ---- constants + routing probs first (small, clears the queue) ----
        A_sb = consts.tile([128, KD * C], bf16)
        nc.sync.dma_start(out=A_sb, in_=a_d[:, :])
        p_sb = gpool.tile([128, N_SUBTOT, E], f32)
        nc.sync.dma_start(out=p_sb, in_=p_d.rearrange("(p n) e -> p n e", p=128))
        I_sb = consts.tile([128, 128], f32)
        nc.sync.dma_start(out=I_sb, in_=i_d[:, :])
        M_raw = consts.tile([E, C], f32)
        nc.sync.dma_start(out=M_raw, in_=m_d[:, :])
        M_sb = consts.tile([E, C], f32r)
        nc.vector.tensor_copy(out=M_sb, in_=M_raw)

        # ---- top-2 gates for the whole core: [128 tok, 16 sub, 8 exp] ----
        m1 = gpool.tile([128, N_SUBTOT, 1], f32)
        nc.vector.tensor_reduce(out=m1, in_=p_sb, axis=AX.X, op=OP.max)
        mlt = gpool.tile([128, N_SUBTOT, E], f32)
        nc.vector.tensor_tensor(
            out=mlt, in0=p_sb, in1=m1.broadcast_to([128, N_SUBTOT, E]), op=OP.is_lt
        )
        pm = gpool.tile([128, N_SUBTOT, E], f32)
        nc.vector.tensor_mul(pm, p_sb, mlt)
        m2 = gpool.tile([128, N_SUBTOT, 1], f32)
        nc.vector.tensor_reduce(out=m2, in_=pm, axis=AX.X, op=OP.max)
        ge2 = gpool.tile([128, N_SUBTOT, E], f32)
        nc.vector.tensor_tensor(
            out=ge2, in0=p_sb, in1=m2.broadcast_to([128, N_SUBTOT, E]), op=OP.is_ge
        )
        gts = gpool.tile([128, N_SUBTOT, E], f32)
        nc.vector.tensor_mul(gts, p_sb, ge2)

        B_issued = [False]

        def issue_ht(g):
            tiles = []
            for h2 in range(2):
                ht = hpool.tile([128, KH, GROUP], bf16, tag="h", name=f"ht{g}_{h2}")
                off = g * KD * GROUP + h2 * KH * GROUP
                nc.sync.dma_start(
                    out=ht,
                    in_=ht_d[:, off : off + KH * GROUP].rearrange(
                        "p (k t) -> p k t", k=KH
                    ),
                )
                tiles.append(ht)
            return tiles

        # issue ALL h loads before any output store enters the (in-order)
        # sync queue — otherwise stores head-of-line block later h streams
        ht_tiles = {0: issue_ht(0)}
        B_sb = consts.tile([C, D], bf16)
        nc.sync.dma_start(out=B_sb, in_=b_d[:, :])
        for g in range(1, N_GROUPS):
            ht_tiles[g] = issue_ht(g)

        def gates_for(g):
            # dense gate matrix G[c, t]: transpose + one-hot expand matmul
            # ([128,1024]-shaped allocs keep the shared pool's slots uniform)
            gt_ps = ps_o.tile([128, 1024], f32, tag="o", name=f"gt{g}")
            for s4 in range(SUB_PER_GROUP):
                s = g * SUB_PER_GROUP + s4
                nc.tensor.transpose(
                    out=gt_ps[:E, s4 * 128 : (s4 + 1) * 128],
                    in_=gts[:, s, :],
                    identity=I_sb,
                )
            gt_sb = gtpool.tile([E, GROUP], f32r, tag="gtsb", name=f"gtsb{g}")
            nc.vector.tensor_copy(out=gt_sb, in_=gt_ps[:E, :GROUP])
            G_ps = ps_o.tile([128, 1024], f32, tag="o", name=f"G{g}")
            nc.tensor.matmul(
                G_ps[:, :GROUP], lhsT=M_sb, rhs=gt_sb, start=True, stop=True
            )
            G_sb = gsbpool.tile([128, GROUP], f32, tag="gsb", name=f"Gsb{g}")
            nc.scalar.copy(out=G_sb, in_=G_ps[:, :GROUP])
            return G_sb

        def stage1(g):
            U_ps = ps_u.tile([128, GROUP], f32, tag="u", name=f"U{g}")
            for k in range(KD):
                nc.tensor.matmul(
                    U_ps,
                    lhsT=A_sb[:, k * C : (k + 1) * C],
                    rhs=ht_tiles[g][k // KH][:, k % KH, :],
                    start=(k == 0),
                    stop=(k == KD - 1),
                )
            return U_ps

        copy_flip = [0]

        def stage2(g, U_ps, G_sb):
            uts = utspool.tile([128, GROUP], bf16, tag="uts", name=f"uts{g}")
            nc.vector.tensor_tensor(out=uts, in0=U_ps, in1=G_sb, op=OP.mult)
            for s4 in range(SUB_PER_GROUP):
                s = g * SUB_PER_GROUP + s4
                o_sb = opool.tile([128, D], bf16, tag="osb", name=f"osb{s}")
                for jh in range(2):
                    o_ps = ps_o.tile([128, 1024], f32, tag="o", name=f"o{s}_{jh}")
                    for j2 in range(2):
                        j = jh * 2 + j2
                        nc.tensor.matmul(
                            o_ps[:, j2 * 512 : (j2 + 1) * 512],
                            lhsT=uts[:, s4 * 128 : (s4 + 1) * 128],
                            rhs=B_sb[:, j * 512 : (j + 1) * 512],
                            start=True,
                            stop=True,
                        )
                    if copy_flip[0] % 2 == 0:
                        nc.vector.tensor_copy(
                            out=o_sb[:, jh * 1024 : (jh + 1) * 1024], in_=o_ps
                        )
                    else:
                        nc.scalar.copy(
                            out=o_sb[:, jh * 1024 : (jh + 1) * 1024], in_=o_ps
                        )
                    copy_flip[0] += 1
                nc.sync.dma_start(out=o_d[s * 128 : (s + 1) * 128, :], in_=o_sb)

        # PE order: gates g0 -> stage1 g0 -> gates g1..g3 (fills the wait
        # for group 1's h stream) -> stage2 g0 -> stage1 g1 -> stage2 g1 ...
        # Monotone logical waits stop the scheduler from hoisting group g+1
        # work above group g's store pipeline (which starves the out DMAs).
        U0 = stage1(0)
        G_sbs = {g: gates_for(g) for g in range(N_GROUPS)}
        U_cur = U0
        for g in range(N_GROUPS):
            tc.tile_set_cur_wait(g + 1)
            stage2(g, U_cur, G_sbs[g])
            if g + 1 < N_GROUPS:
                U_cur = stage1(g + 1)

    if split_waits:
        _split_matmul_waits(nc)
    return nc


def _split_matmul_waits(nc, max_waits=1):
    """Walrus codegen allows only one sync-wait slot on self-loading
    Matmult instructions. Move surplus waits onto a no-op EventSemaphore
    inserted immediately before, same engine — identical semantics."""
    import concourse.mybir as mybir

    n = 0
    for f in nc.m.functions:
        for blk in f.blocks:
            insts = blk.instructions
            new_list = []
            changed = False
            for inst in insts:
                si = inst.sync_info
                if (
                    type(inst).__name__ != "InstEventSemaphore"
                    and si is not None
                    and si.on_wait
                    and len(si.on_wait) > max_waits
                ):
                    surplus = list(si.on_wait[:-max_waits])
                    keep = list(si.on_wait[-max_waits:])
                    for i in range(0, len(surplus), 2):
                        n += 1
                        ev = mybir.InstEventSemaphore(
                            name=f"I-swsplit-{n}", ins=[], outs=[]
                        )
                        ev.engine = inst.engine
                        ev.sync_info = mybir.SyncInfo(
                            on_wait=surplus[i : i + 2], on_update=[]
                        )
                        new_list.append(ev)
                    inst.sync_info = mybir.SyncInfo(
                        on_wait=keep, on_update=list(si.on_update or [])
                    )
                    changed = True
                new_list.append(inst)
            if changed:
                blk.instructions = new_list
    return n


def _host_prep(h, p_L, A, B):
    """Shard tokens across cores; pre-transpose + pre-tile h; helpers."""
    import ml_dtypes

    BF16 = ml_dtypes.bfloat16

    # hT[core][p, g, k, t] = h[core][token g*GROUP+t, d = k*128+p]
    h5 = np.asarray(h, dtype=np.float32).reshape(N_CORES, N_GROUPS, GROUP, KD, 128)
    hT = np.ascontiguousarray(h5.transpose(0, 4, 1, 3, 2)).astype(BF16)
    hT = hT.reshape(N_CORES, 128, N_GROUPS * KD * GROUP)

    # permute p_L rows so partition p holds tokens {n*128+p}: row p*16+n
    p3 = np.asarray(p_L, dtype=np.float32).reshape(N_CORES, N_SUBTOT, 128, E)
    p_perm = np.ascontiguousarray(p3.transpose(0, 2, 1, 3)).reshape(
        N_CORES, T_CORE, E
    )

    # A_cat[d, c] = SCALING * A[e, r, d], pre-arranged [p, k*C + c]
    A_cat = (np.asarray(A, dtype=np.float32) * SCALING).transpose(2, 0, 1).reshape(D, C)
    A_arr = np.ascontiguousarray(
        A_cat.reshape(KD, 128, C).transpose(1, 0, 2).reshape(128, KD * C)
    ).astype(BF16)
    # B_cat[c, d] = B[e, d, r]
    B_cat = (
        np.asarray(B, dtype=np.float32).transpose(0, 2, 1).reshape(C, D).astype(BF16)
    )
    Mexp = np.zeros((E, C), dtype=np.float32)
    for e in range(E):
        Mexp[e, e * R : (e + 1) * R] = 1.0
    Ident = np.eye(128, dtype=np.float32)

    in_maps = []
    for i in range(N_CORES):
        in_maps.append(
            {
                "hT": hT[i],
                "p_perm": p_perm[i],
                "A_cat": A_arr,
                "B_cat": B_cat,
                "Mexp": Mexp,
                "Ident": Ident,
            }
        )
    return in_maps


def _get_nc():
    if "nc" not in _CACHE:
        _CACHE["nc"] = _build_nc()
    return _CACHE["nc"]


def kernel(h, p_L, A, B):
    from concourse.bass_utils import run_bass_kernel_spmd

    nc = _get_nc()
    in_maps = _host_prep(h, p_L, A, B)
    res = run_bass_kernel_spmd(nc, in_maps, core_ids=list(range(N_CORES)))
    out = np.concatenate(
        [np.asarray(res.results[i]["out"]) for i in range(N_CORES)], axis=0
    )
    return out.astype(np.float32).reshape(B_SZ, S_SZ, D)
